# revision 23
# baseline (speedup 1.0000x reference)
"""Trainium2 Bass kernel for nn_LinearRNN (B=16, T=4096, D_in=256, H=512, D_out=256).

  xp = x @ W_in.T                       [B, T, H]
  h_t = xp_t + h_{t-1} @ W_h.T          (W_h is diagonal -> elementwise scan)
  out = hs @ W_out.T                    [B, T, D_out]

Strategy: batch data-parallel over 8 cores (2 batch rows per core), bf16
matmul operands (1 cyc/row on PE, rel-err ~6.5e-3), bf16 output staging
upcast on the host. Per core (~62.7us vs the ~54.6us PE roofline):
  - host pre-transposes x to [b, d, t]; weights pre-transposed likewise.
  - packed "head" DMAs carry the first W_in slices + the first x columns of
    each batch row so the first matmuls start ~3.6us in (one DMA chain is
    ~2.2us of fixed HWDGE/DGE/sem latency, so the criticals are packed into
    one transfer; the f32 decay columns ride inside head1's bf16 transfer
    as raw bytes and are read back via an AP bitcast; head2 rides the Pool
    SWDGE path to dodge the shared HWDGE device, which serializes DMA
    issue 625ns apart),
  - dependency-free "prime" matmuls (into the first xp tile, which the first
    real matmul then overwrites) start the PE p-state ramp clock at t~0,
  - per (batch, t-chunk): matmul1 on TensorE -> PSUM, VectorE
    tensor_tensor_scan runs the recurrence along t (carry chained across
    chunks), matmul2 contracts back to d_out, ScalarE copies PSUM->SBUF
    into per-group staging, SP DMAs each finished group out (one DMA covers
    both o-blocks),
  - chunks are small (128/256) at both ends of the t-range to shrink the
    pipeline fill/drain, 512 in steady state; b0/b1 chunks interleave.
  - the scan-init zero lives in an extra dcols column so the tile framework
    emits no const-materialization prologue barrier.
"""
from contextlib import ExitStack

import numpy as np

import concourse.bass as bass
import concourse.mybir as mybir
import concourse.tile as tile
from concourse import bacc
from concourse.bass_utils import run_bass_kernel_spmd

B, T, D_IN, HID, D_OUT = 16, 4096, 256, 512, 256
NCORES = 8
BPC = B // NCORES          # batch rows per core
ND = D_IN // 128           # 2  d-blocks
NH = HID // 128            # 4  h-blocks
NO = D_OUT // 128          # 2  o-blocks

MODE_DEFAULT = "bf16"

# schedule/tuning knobs (read by _build; cache key includes them)
CFG = dict(
    # per-batch chunk sizes along t (must sum to T)
    chunks=(256, 256, 512, 512, 512, 512, 512, 512, 256, 256),
    # chunk-index groups per output DMA (one staging tile + one DMA each)
    out_groups=((0,), (1,), (2,), (3,), (4,), (5,), (6,), (7,), (8,), (9,)),
    primes=21,             # number of 128-wide prime matmuls
    out_bf16=True,         # stage+DMA outputs as bf16, upcast on host
    xp_bufs=5, op_bufs=3, hs_bufs=16,
    head_cols=256,         # x columns per batch row carried by the head DMAs
    x_second=1024,         # x cols covered by the second piece (from head_cols)
    x_piece=1024,          # bulk x piece size
    pool_scan_units=0,     # first N units: odd-hblk scans go to GPSIMD
    dc_on_sp=False,        # dc as first SP DMA instead of Pool SWDGE
    tail_split=False,      # final unit: stage2 in two col-halves, 2 DMAs
    wi_in_head=False,      # carry all of w_in in head1 (else h23 separate)
)

_cache: dict = {}


def _chunk_plan():
    chunks = []
    start = 0
    for sz in CFG["chunks"]:
        chunks.append((start, sz))
        start += sz
    assert start == T, f"chunks sum {start} != {T}"
    return chunks


def _build(mode: str) -> bass.Bass:
    f32 = mybir.dt.float32
    dt_in = {"bf16": mybir.dt.bfloat16, "f32r": mybir.dt.float32r}.get(
        mode, f32)
    chunks = _chunk_plan()
    groups = CFG["out_groups"]
    assert tuple(sorted(i for g in groups for i in g)) == tuple(range(len(chunks)))
    group_of = {ci: gi for gi, g in enumerate(groups) for ci in g}
    HB = CFG["head_cols"]
    # head1 cols: wi_d0_h01 | wi_d1_h01 | xb0_d0[0:HB] | xb0_d1[0:HB] |
    #             dcols' raw f32 bytes as 2*(NH+1) bf16 cols
    WIH = 1024 if CFG["wi_in_head"] else 512
    H1_COLS = WIH + 2 * HB + 2 * (NH + 1)
    # head2 cols: xb1_d0[0:HB] | xb1_d1[0:HB]
    H2_COLS = 2 * HB

    nc = bacc.Bacc(None, target_bir_lowering=False)

    head1 = nc.declare_dram_parameter("head1", [128, H1_COLS], dt_in,
                                      isOutput=False)
    head2 = nc.declare_dram_parameter("head2", [128, H2_COLS], dt_in,
                                      isOutput=False)
    xT = nc.declare_dram_parameter("xT", [BPC, D_IN, T], dt_in, isOutput=False)
    w_inT = nc.declare_dram_parameter("w_inT", [D_IN, HID], dt_in,
                                      isOutput=False)
    w_outT = nc.declare_dram_parameter("w_outT", [HID, D_OUT], dt_in,
                                       isOutput=False)
    dt_out = dt_in if CFG["out_bf16"] and mode == "bf16" else f32
    out = nc.declare_dram_parameter("out", [BPC, D_OUT, T], dt_out,
                                    isOutput=True)

    with tile.TileContext(nc) as tc, ExitStack() as ctx:
        const_pool = ctx.enter_context(tc.tile_pool(name="const", bufs=1))
        x_pool = ctx.enter_context(tc.tile_pool(name="xt", bufs=BPC * ND))
        o_pool = ctx.enter_context(tc.tile_pool(name="ot", bufs=6))
        hs_pool = ctx.enter_context(tc.tile_pool(name="hs", bufs=CFG["hs_bufs"]))
        xp_psum = ctx.enter_context(
            tc.tile_pool(name="xp", bufs=CFG["xp_bufs"],
                         space=bass.MemorySpace.PSUM))
        op_psum = ctx.enter_context(
            tc.tile_pool(name="op", bufs=CFG["op_bufs"],
                         space=bass.MemorySpace.PSUM))

        # ---- prime matmuls: minimal data deps (one tiny DVE memset); they
        # start the PE p-state ramp clock at t~0. They write into the first
        # real xp tile (cols <=128), which the first real matmul overwrites
        # (start=True) before the scan reads it.
        junk = const_pool.tile([128, 128], dt_in, tag="junk")
        nc.vector.memset(junk[:], 0.0)
        prime_xp = xp_psum.tile([128, 512], f32, name="xp", tag="xp")
        for _ in range(CFG["primes"]):
            nc.tensor.matmul(prime_xp[:, :128], junk[:], junk[:],
                             start=True, stop=True)

        # ---- SBUF tiles
        hd1 = const_pool.tile([128, H1_COLS], dt_in, tag="head1")
        hd2 = const_pool.tile([128, H2_COLS], dt_in, tag="head2")
        wi = const_pool.tile([128, 512], dt_in, tag="wi")       # h23: d0|d1
        wo = const_pool.tile([128, NH * D_OUT], dt_in, tag="wo")  # h-major
        # decay columns: raw f32 bytes shipped inside head1's bf16 transfer
        dcoff = (WIH + 2 * HB) // 2
        dc = hd1[:].bitcast(f32)[:, dcoff:dcoff + NH + 1]
        xt = {}
        for b in range(BPC):
            for dblk in range(ND):
                xt[(b, dblk)] = x_pool.tile([128, T], dt_in, name="xt",
                                            tag="xt")

        def wi_ap(dblk, hblk):
            if CFG["wi_in_head"]:
                return hd1[:, dblk * 512 + hblk * 128:
                           dblk * 512 + (hblk + 1) * 128]
            if hblk < 2:
                return hd1[:, dblk * 256 + hblk * 128:
                           dblk * 256 + (hblk + 1) * 128]
            return wi[:, dblk * 256 + (hblk - 2) * 128:
                      dblk * 256 + (hblk - 1) * 128]

        def x_ap(b, dblk, t0, csz):
            if t0 + csz <= HB:
                hd = (hd1, hd2)[b]
                off = (WIH, 0)[b] + dblk * HB
                return hd[:, off + t0: off + t0 + csz]
            return xt[(b, dblk)][:, t0:t0 + csz]

        # ---- DMA program (emission order = issue order on the SP queue)
        nc.sync.dma_start(hd1[:], head1[:])
        # head2 goes via Pool SWDGE: a parallel issue path that skips the
        # HWDGE device, which serializes SP-queue DMAs 625ns apart
        nc.gpsimd.dma_start(hd2[:], head2[:])
        if not CFG["wi_in_head"]:
            # w_in h-blocks 2/3: rows (dblk*128..), cols [256:512] -> wi d0|d1
            nc.sync.dma_start(
                wi[:].rearrange("p (d h) -> p d h", d=ND),
                w_inT[:].rearrange("(d p) h -> p d h", d=ND)[:, :, 256:])
        # w_out in two halves (hblk 0/1, then 2/3)
        for half in range(2):
            sl = slice(half * 2 * D_OUT, (half + 1) * 2 * D_OUT)
            nc.sync.dma_start(
                wo[:, sl].rearrange("p (h o) -> p h o", h=2),
                w_outT[:].rearrange("(h p) o -> p h o", h=NH)[:, half * 2:half * 2 + 2, :])
        # second x pieces [HB:XS], b0 then b1
        XS = CFG["x_second"]
        for b in range(BPC):
            for dblk in range(ND):
                nc.sync.dma_start(xt[(b, dblk)][:, HB:XS],
                                  xT[b, dblk * 128:(dblk + 1) * 128, HB:XS])
        # bulk x pieces, round-robin b0/b1
        XP = CFG["x_piece"]
        starts = {0: XS, 1: XS}
        while starts[0] < T or starts[1] < T:
            for b in range(BPC):
                s = starts[b]
                if s >= T:
                    continue
                e = min(s + XP, T)
                for dblk in range(ND):
                    nc.sync.dma_start(xt[(b, dblk)][:, s:e],
                                      xT[b, dblk * 128:(dblk + 1) * 128, s:e])
                starts[b] = e

        # ---- compute pipeline
        units = [(b, ci) for ci in range(len(chunks)) for b in range(BPC)]
        prev_hs = {}
        ot_tiles = {}

        fill_xp = {}

        def stage1(b, ci):
            t0, csz = chunks[ci]
            for hblk in range(NH):
                # ci==0 chunks are 256 wide: both batches share one PSUM bank
                # per hblk at different column offsets (halves fill-phase
                # bank pressure; subtile deps keep the scans independent)
                if ci == 0 and csz <= 256:
                    if hblk not in fill_xp:
                        fill_xp[hblk] = (prime_xp if hblk == 0 else
                                         xp_psum.tile([128, 512], f32,
                                                      name="xp", tag="xp"))
                    xp = fill_xp[hblk][:, b * 256: b * 256 + csz]
                elif (b, ci, hblk) == (0, 0, 0):
                    xp = prime_xp[:, :csz]
                else:
                    xp = xp_psum.tile([128, 512], f32,
                                      name="xp", tag="xp")[:, :csz]
                for dblk in range(ND):
                    nc.tensor.matmul(
                        xp,
                        wi_ap(dblk, hblk),
                        x_ap(b, dblk, t0, csz),
                        start=(dblk == 0), stop=(dblk == ND - 1))
                hs = hs_pool.tile([128, 512], dt_in, name="hs", tag="hs")
                init = (dc[:, NH:NH + 1] if ci == 0
                        else prev_hs[(b, ci - 1, hblk)][:, chunks[ci - 1][1] - 1:
                                                        chunks[ci - 1][1]])
                unit_idx = units.index((b, ci))
                eng = (nc.gpsimd if unit_idx < CFG["pool_scan_units"]
                       and hblk % 2 == 1 else nc.vector)
                eng.tensor_tensor_scan(
                    hs[:, :csz], dc[:, hblk:hblk + 1].to_broadcast((128, csz)),
                    xp, init,
                    op0=mybir.AluOpType.mult, op1=mybir.AluOpType.add)
                prev_hs[(b, ci, hblk)] = hs

        def stage2(b, ci):
            t0, csz = chunks[ci]
            gi = group_of[ci]
            g0 = chunks[groups[gi][0]][0]
            gsz = sum(chunks[i][1] for i in groups[gi])
            if (b, gi) not in ot_tiles:
                ot_tiles[(b, gi)] = o_pool.tile([128, 2 * gsz], dt_out,
                                                name="ot", tag="ot")
            ot = ot_tiles[(b, gi)]
            last = (b, ci) == units[-1]
            if last and CFG["tail_split"] and csz % 2 == 0 and gsz == csz:
                # final unit: two column-halves, each with its own copy pair
                # (ACT for oblk0, DVE for oblk1) and its own out DMA, so the
                # drain chain hangs off a half-width copy+transfer
                h2 = csz // 2
                for half in range(2):
                    cs = slice(half * h2, (half + 1) * h2)
                    for oblk in range(NO):
                        op = op_psum.tile([128, 512], f32, name="op", tag="op")
                        for hblk in range(NH):
                            nc.tensor.matmul(
                                op[:, :h2],
                                wo[:, hblk * D_OUT + oblk * 128:
                                   hblk * D_OUT + (oblk + 1) * 128],
                                prev_hs[(b, ci, hblk)][:, cs],
                                start=(hblk == 0), stop=(hblk == NH - 1))
                        dst = ot[:, oblk * csz + half * h2:
                                 oblk * csz + (half + 1) * h2]
                        if oblk == 1:
                            nc.vector.tensor_scalar(
                                dst, op[:, :h2], 0.0, None,
                                op0=mybir.AluOpType.add)
                        else:
                            nc.scalar.copy(dst, op[:, :h2])
                    nc.sync.dma_start(
                        out[b].rearrange("(o p) t -> p o t", o=NO)
                        [:, :, t0 + half * h2: t0 + (half + 1) * h2],
                        ot[:].rearrange("p (o t) -> p o t", o=NO)[:, :, cs])
                return
            for oblk in range(NO):
                op = op_psum.tile([128, 512], f32, name="op", tag="op")
                for hblk in range(NH):
                    nc.tensor.matmul(
                        op[:, :csz],
                        wo[:, hblk * D_OUT + oblk * 128:
                           hblk * D_OUT + (oblk + 1) * 128],
                        prev_hs[(b, ci, hblk)][:, :csz],
                        start=(hblk == 0), stop=(hblk == NH - 1))
                dst = ot[:, oblk * gsz + (t0 - g0):
                         oblk * gsz + (t0 - g0) + csz]
                if last and oblk == 1:
                    nc.vector.tensor_scalar(
                        dst, op[:, :csz], 0.0, None,
                        op0=mybir.AluOpType.add)
                else:
                    nc.scalar.copy(dst, op[:, :csz])
            if ci == groups[gi][-1]:
                nc.sync.dma_start(
                    out[b].rearrange("(o p) t -> p o t", o=NO)[:, :, g0:g0 + gsz],
                    ot[:].rearrange("p (o t) -> p o t", o=NO))

        stage1(*units[0])
        for k in range(len(units) - 1):
            stage1(*units[k + 1])
            stage2(*units[k])
        stage2(*units[-1])

    nc.compile()
    return nc


def _prep_inputs(x, W_in, W_h, W_out, mode: str):
    npdt = np.float32
    if mode == "bf16":
        import ml_dtypes
        npdt = ml_dtypes.bfloat16
    x = np.asarray(x, np.float32)
    xT = np.ascontiguousarray(np.transpose(x, (0, 2, 1))).astype(npdt)
    w_inT = np.ascontiguousarray(np.asarray(W_in, np.float32).T).astype(npdt)
    w_outT = np.ascontiguousarray(np.asarray(W_out, np.float32).T).astype(npdt)
    d = np.ascontiguousarray(np.diagonal(np.asarray(W_h, np.float32)))
    dcols = np.zeros((128, NH + 1), dtype=np.float32)
    dcols[:, :NH] = d.reshape(NH, 128).T
    import ml_dtypes as _mld
    dcols_bytes = np.ascontiguousarray(dcols).view(_mld.bfloat16)
    HB = CFG["head_cols"]
    in_maps = []
    for c in range(NCORES):
        xc = xT[c * BPC:(c + 1) * BPC]       # [BPC, D_IN, T]
        wn = 512 if CFG["wi_in_head"] else 256
        head1 = np.concatenate(
            [w_inT[0:128, 0:wn], w_inT[128:256, 0:wn],
             xc[0, 0:128, 0:HB], xc[0, 128:256, 0:HB],
             dcols_bytes], axis=1)
        head2 = np.concatenate(
            [xc[1, 0:128, 0:HB], xc[1, 128:256, 0:HB]], axis=1)
        in_maps.append({
            "head1": np.ascontiguousarray(head1),
            "head2": np.ascontiguousarray(head2),
            "xT": np.ascontiguousarray(xc),
            "w_inT": w_inT,
            "w_outT": w_outT,
        })
    return in_maps


def _get_nc(mode: str = MODE_DEFAULT):
    key = (mode, str(sorted(CFG.items())))
    if key not in _cache:
        _cache[key] = _build(mode)
    return _cache[key]


def _run(x, W_in, W_h, W_out, mode: str = MODE_DEFAULT, **spmd_kwargs):
    nc = _get_nc(mode)
    in_maps = _prep_inputs(x, W_in, W_h, W_out, mode)
    res = run_bass_kernel_spmd(nc, in_maps, list(range(NCORES)), **spmd_kwargs)
    parts = [np.transpose(np.asarray(res.results[c]["out"]), (0, 2, 1))
             for c in range(NCORES)]
    full = np.concatenate(parts, axis=0).astype(np.float32)
    return full, res


def kernel(x, W_in, W_h, W_out):
    out, _ = _run(x, W_in, W_h, W_out)
    return out


# revision 24
# speedup vs baseline: 1.0099x; 1.0099x over previous
"""Trainium2 Bass kernel for nn_LinearRNN (B=16, T=4096, D_in=256, H=512, D_out=256).

  xp = x @ W_in.T                       [B, T, H]
  h_t = xp_t + h_{t-1} @ W_h.T          (W_h is diagonal -> elementwise scan)
  out = hs @ W_out.T                    [B, T, D_out]

Strategy: batch data-parallel over 8 cores (2 batch rows per core), bf16
matmul operands (1 cyc/row on PE, rel-err ~6.5e-3), bf16 output staging
upcast on the host. Per core (~62.7us vs the ~54.6us PE roofline):
  - host pre-transposes x to [b, d, t]; weights pre-transposed likewise.
  - packed "head" DMAs carry the first W_in slices + the first x columns of
    each batch row so the first matmuls start ~3.6us in (one DMA chain is
    ~2.2us of fixed HWDGE/DGE/sem latency, so the criticals are packed into
    one transfer; the f32 decay columns ride inside head1's bf16 transfer
    as raw bytes and are read back via an AP bitcast; head2 rides the Pool
    SWDGE path to dodge the shared HWDGE device, which serializes DMA
    issue 625ns apart),
  - dependency-free "prime" matmuls (into the first xp tile, which the first
    real matmul then overwrites) start the PE p-state ramp clock at t~0,
  - per (batch, t-chunk): matmul1 on TensorE -> PSUM, VectorE
    tensor_tensor_scan runs the recurrence along t (carry chained across
    chunks), matmul2 contracts back to d_out, ScalarE copies PSUM->SBUF
    into per-group staging, SP DMAs each finished group out (one DMA covers
    both o-blocks),
  - chunks are small (128/256) at both ends of the t-range to shrink the
    pipeline fill/drain, 512 in steady state; b0/b1 chunks interleave.
  - the scan-init zero lives in an extra dcols column so the tile framework
    emits no const-materialization prologue barrier.
"""
from contextlib import ExitStack

import numpy as np

import concourse.bass as bass
import concourse.mybir as mybir
import concourse.tile as tile
from concourse import bacc
from concourse.bass_utils import run_bass_kernel_spmd

B, T, D_IN, HID, D_OUT = 16, 4096, 256, 512, 256
NCORES = 8
BPC = B // NCORES          # batch rows per core
ND = D_IN // 128           # 2  d-blocks
NH = HID // 128            # 4  h-blocks
NO = D_OUT // 128          # 2  o-blocks

MODE_DEFAULT = "bf16"

# schedule/tuning knobs (read by _build; cache key includes them)
CFG = dict(
    # per-batch chunk sizes along t (must sum to T)
    chunks=(256, 256, 512, 512, 512, 512, 512, 512, 256, 256),
    # chunk-index groups per output DMA (one staging tile + one DMA each)
    out_groups=((0,), (1,), (2,), (3,), (4,), (5,), (6,), (7,), (8,), (9,)),
    primes=21,             # number of 128-wide prime matmuls
    out_bf16=True,         # stage+DMA outputs as bf16, upcast on host
    xp_bufs=5, op_bufs=3, hs_bufs=16,
    head_cols=256,         # x columns per batch row carried by the head DMAs
    x_second=1024,         # x cols covered by the second piece (from head_cols)
    x_piece=1024,          # bulk x piece size
    pool_scan_units=0,     # first N units: odd-hblk scans go to GPSIMD
    dc_on_sp=False,        # dc as first SP DMA instead of Pool SWDGE
    tail_split=False,      # final unit: stage2 in two col-halves, 2 DMAs
    wi_in_head=False,      # carry all of w_in in head1 (else h23 separate)
)

_cache: dict = {}


def _chunk_plan():
    chunks = []
    start = 0
    for sz in CFG["chunks"]:
        chunks.append((start, sz))
        start += sz
    assert start == T, f"chunks sum {start} != {T}"
    return chunks


def _build(mode: str) -> bass.Bass:
    f32 = mybir.dt.float32
    dt_in = {"bf16": mybir.dt.bfloat16, "f32r": mybir.dt.float32r}.get(
        mode, f32)
    chunks = _chunk_plan()
    groups = CFG["out_groups"]
    assert tuple(sorted(i for g in groups for i in g)) == tuple(range(len(chunks)))
    group_of = {ci: gi for gi, g in enumerate(groups) for ci in g}
    HB = CFG["head_cols"]
    # head1 cols: wi_d0_h01 | wi_d1_h01 | xb0_d0[0:HB] | xb0_d1[0:HB] |
    #             dcols' raw f32 bytes as 2*(NH+1) bf16 cols
    WIH = 1024 if CFG["wi_in_head"] else 512
    H1_COLS = WIH + 2 * HB + 2 * (NH + 1)
    # head2 cols: xb1_d0[0:HB] | xb1_d1[0:HB]
    H2_COLS = 2 * HB

    nc = bacc.Bacc(None, target_bir_lowering=False)

    head1 = nc.declare_dram_parameter("head1", [128, H1_COLS], dt_in,
                                      isOutput=False)
    head2 = nc.declare_dram_parameter("head2", [128, H2_COLS], dt_in,
                                      isOutput=False)
    xT = nc.declare_dram_parameter("xT", [BPC, D_IN, T], dt_in, isOutput=False)
    w_inT = nc.declare_dram_parameter("w_inT", [D_IN, HID], dt_in,
                                      isOutput=False)
    w_outT = nc.declare_dram_parameter("w_outT", [HID, D_OUT], dt_in,
                                       isOutput=False)
    dt_out = dt_in if CFG["out_bf16"] and mode == "bf16" else f32
    out = nc.declare_dram_parameter("out", [BPC, D_OUT, T], dt_out,
                                    isOutput=True)

    with tile.TileContext(nc) as tc, ExitStack() as ctx:
        const_pool = ctx.enter_context(tc.tile_pool(name="const", bufs=1))
        x_pool = ctx.enter_context(tc.tile_pool(name="xt", bufs=BPC * ND))
        o_pool = ctx.enter_context(tc.tile_pool(name="ot", bufs=6))
        hs_pool = ctx.enter_context(tc.tile_pool(name="hs", bufs=CFG["hs_bufs"]))
        xp_psum = ctx.enter_context(
            tc.tile_pool(name="xp", bufs=CFG["xp_bufs"],
                         space=bass.MemorySpace.PSUM))
        op_psum = ctx.enter_context(
            tc.tile_pool(name="op", bufs=CFG["op_bufs"],
                         space=bass.MemorySpace.PSUM))

        # ---- prime matmuls: minimal data deps (one tiny DVE memset); they
        # start the PE p-state ramp clock at t~0. They write into the first
        # real xp tile (cols <=128), which the first real matmul overwrites
        # (start=True) before the scan reads it.
        junk = const_pool.tile([128, 128], dt_in, tag="junk")
        nc.vector.memset(junk[:], 0.0)
        prime_xp = xp_psum.tile([128, 512], f32, name="xp", tag="xp")
        for _ in range(CFG["primes"]):
            nc.tensor.matmul(prime_xp[:, :128], junk[:], junk[:],
                             start=True, stop=True)

        # ---- SBUF tiles
        hd1 = const_pool.tile([128, H1_COLS], dt_in, tag="head1")
        hd2 = const_pool.tile([128, H2_COLS], dt_in, tag="head2")
        wi = const_pool.tile([128, 512], dt_in, tag="wi")       # h23: d0|d1
        wo = const_pool.tile([128, NH * D_OUT], dt_in, tag="wo")  # h-major
        # decay columns: raw f32 bytes shipped inside head1's bf16 transfer
        dcoff = (WIH + 2 * HB) // 2
        dc = hd1[:].bitcast(f32)[:, dcoff:dcoff + NH + 1]
        xt = {}
        for b in range(BPC):
            for dblk in range(ND):
                xt[(b, dblk)] = x_pool.tile([128, T], dt_in, name="xt",
                                            tag="xt")

        def wi_ap(dblk, hblk):
            if CFG["wi_in_head"]:
                return hd1[:, dblk * 512 + hblk * 128:
                           dblk * 512 + (hblk + 1) * 128]
            if hblk < 2:
                return hd1[:, dblk * 256 + hblk * 128:
                           dblk * 256 + (hblk + 1) * 128]
            return wi[:, dblk * 256 + (hblk - 2) * 128:
                      dblk * 256 + (hblk - 1) * 128]

        def x_ap(b, dblk, t0, csz):
            if t0 + csz <= HB:
                hd = (hd1, hd2)[b]
                off = (WIH, 0)[b] + dblk * HB
                return hd[:, off + t0: off + t0 + csz]
            return xt[(b, dblk)][:, t0:t0 + csz]

        # ---- DMA program (emission order = issue order on the SP queue)
        nc.sync.dma_start(hd1[:], head1[:])
        # head2 goes via Pool SWDGE: a parallel issue path that skips the
        # HWDGE device, which serializes SP-queue DMAs 625ns apart
        nc.gpsimd.dma_start(hd2[:], head2[:])
        if not CFG["wi_in_head"]:
            # w_in h-blocks 2/3: rows (dblk*128..), cols [256:512] -> wi d0|d1
            nc.sync.dma_start(
                wi[:].rearrange("p (d h) -> p d h", d=ND),
                w_inT[:].rearrange("(d p) h -> p d h", d=ND)[:, :, 256:])
        # w_out in two halves (hblk 0/1, then 2/3)
        for half in range(2):
            sl = slice(half * 2 * D_OUT, (half + 1) * 2 * D_OUT)
            nc.sync.dma_start(
                wo[:, sl].rearrange("p (h o) -> p h o", h=2),
                w_outT[:].rearrange("(h p) o -> p h o", h=NH)[:, half * 2:half * 2 + 2, :])
        # second x pieces [HB:XS], b0 then b1
        XS = CFG["x_second"]
        for b in range(BPC):
            for dblk in range(ND):
                nc.sync.dma_start(xt[(b, dblk)][:, HB:XS],
                                  xT[b, dblk * 128:(dblk + 1) * 128, HB:XS])
        # bulk x pieces, round-robin b0/b1
        XP = CFG["x_piece"]
        starts = {0: XS, 1: XS}
        while starts[0] < T or starts[1] < T:
            for b in range(BPC):
                s = starts[b]
                if s >= T:
                    continue
                e = min(s + XP, T)
                for dblk in range(ND):
                    nc.sync.dma_start(xt[(b, dblk)][:, s:e],
                                      xT[b, dblk * 128:(dblk + 1) * 128, s:e])
                starts[b] = e

        # ---- compute pipeline
        units = [(b, ci) for ci in range(len(chunks)) for b in range(BPC)]
        prev_hs = {}
        ot_tiles = {}

        fill_xp = {}

        def stage1(b, ci):
            t0, csz = chunks[ci]
            for hblk in range(NH):
                # ci==0 chunks are 256 wide: both batches share one PSUM bank
                # per hblk at different column offsets (halves fill-phase
                # bank pressure; subtile deps keep the scans independent)
                if ci == 0 and csz <= 256:
                    if hblk not in fill_xp:
                        fill_xp[hblk] = (prime_xp if hblk == 0 else
                                         xp_psum.tile([128, 512], f32,
                                                      name="xp", tag="xp"))
                    xp = fill_xp[hblk][:, b * 256: b * 256 + csz]
                elif (b, ci, hblk) == (0, 0, 0):
                    xp = prime_xp[:, :csz]
                else:
                    xp = xp_psum.tile([128, 512], f32,
                                      name="xp", tag="xp")[:, :csz]
                for dblk in range(ND):
                    nc.tensor.matmul(
                        xp,
                        wi_ap(dblk, hblk),
                        x_ap(b, dblk, t0, csz),
                        start=(dblk == 0), stop=(dblk == ND - 1))
                hs = hs_pool.tile([128, 512], dt_in, name="hs", tag="hs")
                init = (dc[:, NH:NH + 1] if ci == 0
                        else prev_hs[(b, ci - 1, hblk)][:, chunks[ci - 1][1] - 1:
                                                        chunks[ci - 1][1]])
                unit_idx = units.index((b, ci))
                eng = (nc.gpsimd if unit_idx < CFG["pool_scan_units"]
                       and hblk % 2 == 1 else nc.vector)
                eng.tensor_tensor_scan(
                    hs[:, :csz], dc[:, hblk:hblk + 1].to_broadcast((128, csz)),
                    xp, init,
                    op0=mybir.AluOpType.mult, op1=mybir.AluOpType.add)
                prev_hs[(b, ci, hblk)] = hs

        def stage2(b, ci):
            t0, csz = chunks[ci]
            gi = group_of[ci]
            g0 = chunks[groups[gi][0]][0]
            gsz = sum(chunks[i][1] for i in groups[gi])
            if (b, gi) not in ot_tiles:
                ot_tiles[(b, gi)] = o_pool.tile([128, 2 * gsz], dt_out,
                                                name="ot", tag="ot")
            ot = ot_tiles[(b, gi)]
            last = (b, ci) == units[-1]
            if last and CFG["tail_split"] and csz % 2 == 0 and gsz == csz:
                # final unit: two column-halves, each with its own copy pair
                # (ACT for oblk0, DVE for oblk1) and its own out DMA, so the
                # drain chain hangs off a half-width copy+transfer
                h2 = csz // 2
                for half in range(2):
                    cs = slice(half * h2, (half + 1) * h2)
                    for oblk in range(NO):
                        op = op_psum.tile([128, 512], f32, name="op", tag="op")
                        for hblk in range(NH):
                            nc.tensor.matmul(
                                op[:, :h2],
                                wo[:, hblk * D_OUT + oblk * 128:
                                   hblk * D_OUT + (oblk + 1) * 128],
                                prev_hs[(b, ci, hblk)][:, cs],
                                start=(hblk == 0), stop=(hblk == NH - 1))
                        dst = ot[:, oblk * csz + half * h2:
                                 oblk * csz + (half + 1) * h2]
                        if oblk == 1:
                            nc.vector.tensor_scalar(
                                dst, op[:, :h2], 0.0, None,
                                op0=mybir.AluOpType.add)
                        else:
                            nc.scalar.copy(dst, op[:, :h2])
                    nc.sync.dma_start(
                        out[b].rearrange("(o p) t -> p o t", o=NO)
                        [:, :, t0 + half * h2: t0 + (half + 1) * h2],
                        ot[:].rearrange("p (o t) -> p o t", o=NO)[:, :, cs])
                return
            for oblk in range(NO):
                op = op_psum.tile([128, 512], f32, name="op", tag="op")
                for hblk in range(NH):
                    nc.tensor.matmul(
                        op[:, :csz],
                        wo[:, hblk * D_OUT + oblk * 128:
                           hblk * D_OUT + (oblk + 1) * 128],
                        prev_hs[(b, ci, hblk)][:, :csz],
                        start=(hblk == 0), stop=(hblk == NH - 1))
                dst = ot[:, oblk * gsz + (t0 - g0):
                         oblk * gsz + (t0 - g0) + csz]
                if last and oblk == 1:
                    nc.vector.tensor_scalar(
                        dst, op[:, :csz], 0.0, None,
                        op0=mybir.AluOpType.add)
                else:
                    nc.scalar.copy(dst, op[:, :csz])
            if ci == groups[gi][-1]:
                nc.sync.dma_start(
                    out[b].rearrange("(o p) t -> p o t", o=NO)[:, :, g0:g0 + gsz],
                    ot[:].rearrange("p (o t) -> p o t", o=NO))

        stage1(*units[0])
        for k in range(len(units) - 1):
            stage1(*units[k + 1])
            stage2(*units[k])
        stage2(*units[-1])

    # Strip the framework's entry-block prologue: four const-AP memsets
    # (f32 0/1, bf16 1, uint8 127 -- none are read by this program) and the
    # all-engine barrier that orders them before the kernel. Engines then
    # branch straight into the tile block ~600ns earlier. The end-of-program
    # drain barrier is untouched (it guarantees output DMAs complete).
    entry = nc.m.functions[0].blocks[0]
    entry.instructions = [
        inst for inst in entry.instructions
        if inst.opcode not in ("Memset", "Drain", "EventSemaphore")
    ]
    nc.compile()
    return nc


def _prep_inputs(x, W_in, W_h, W_out, mode: str):
    npdt = np.float32
    if mode == "bf16":
        import ml_dtypes
        npdt = ml_dtypes.bfloat16
    x = np.asarray(x, np.float32)
    xT = np.ascontiguousarray(np.transpose(x, (0, 2, 1))).astype(npdt)
    w_inT = np.ascontiguousarray(np.asarray(W_in, np.float32).T).astype(npdt)
    w_outT = np.ascontiguousarray(np.asarray(W_out, np.float32).T).astype(npdt)
    d = np.ascontiguousarray(np.diagonal(np.asarray(W_h, np.float32)))
    dcols = np.zeros((128, NH + 1), dtype=np.float32)
    dcols[:, :NH] = d.reshape(NH, 128).T
    import ml_dtypes as _mld
    dcols_bytes = np.ascontiguousarray(dcols).view(_mld.bfloat16)
    HB = CFG["head_cols"]
    in_maps = []
    for c in range(NCORES):
        xc = xT[c * BPC:(c + 1) * BPC]       # [BPC, D_IN, T]
        wn = 512 if CFG["wi_in_head"] else 256
        head1 = np.concatenate(
            [w_inT[0:128, 0:wn], w_inT[128:256, 0:wn],
             xc[0, 0:128, 0:HB], xc[0, 128:256, 0:HB],
             dcols_bytes], axis=1)
        head2 = np.concatenate(
            [xc[1, 0:128, 0:HB], xc[1, 128:256, 0:HB]], axis=1)
        in_maps.append({
            "head1": np.ascontiguousarray(head1),
            "head2": np.ascontiguousarray(head2),
            "xT": np.ascontiguousarray(xc),
            "w_inT": w_inT,
            "w_outT": w_outT,
        })
    return in_maps


def _get_nc(mode: str = MODE_DEFAULT):
    key = (mode, str(sorted(CFG.items())))
    if key not in _cache:
        _cache[key] = _build(mode)
    return _cache[key]


def _run(x, W_in, W_h, W_out, mode: str = MODE_DEFAULT, **spmd_kwargs):
    nc = _get_nc(mode)
    in_maps = _prep_inputs(x, W_in, W_h, W_out, mode)
    res = run_bass_kernel_spmd(nc, in_maps, list(range(NCORES)), **spmd_kwargs)
    parts = [np.transpose(np.asarray(res.results[c]["out"]), (0, 2, 1))
             for c in range(NCORES)]
    full = np.concatenate(parts, axis=0).astype(np.float32)
    return full, res


def kernel(x, W_in, W_h, W_out):
    out, _ = _run(x, W_in, W_h, W_out)
    return out


# revision 25
# speedup vs baseline: 1.0115x; 1.0015x over previous
"""Trainium2 Bass kernel for nn_LinearRNN (B=16, T=4096, D_in=256, H=512, D_out=256).

  xp = x @ W_in.T                       [B, T, H]
  h_t = xp_t + h_{t-1} @ W_h.T          (W_h is diagonal -> elementwise scan)
  out = hs @ W_out.T                    [B, T, D_out]

Strategy: batch data-parallel over 8 cores (2 batch rows per core), bf16
matmul operands (1 cyc/row on PE, rel-err ~6.5e-3), bf16 output staging
upcast on the host. Per core (~62.7us vs the ~54.6us PE roofline):
  - host pre-transposes x to [b, d, t]; weights pre-transposed likewise.
  - packed "head" DMAs carry the first W_in slices + the first x columns of
    each batch row so the first matmuls start ~3.6us in (one DMA chain is
    ~2.2us of fixed HWDGE/DGE/sem latency, so the criticals are packed into
    one transfer; the f32 decay columns ride inside head1's bf16 transfer
    as raw bytes and are read back via an AP bitcast; head2 rides the Pool
    SWDGE path to dodge the shared HWDGE device, which serializes DMA
    issue 625ns apart),
  - dependency-free "prime" matmuls (into the first xp tile, which the first
    real matmul then overwrites) start the PE p-state ramp clock at t~0,
  - per (batch, t-chunk): matmul1 on TensorE -> PSUM, VectorE
    tensor_tensor_scan runs the recurrence along t (carry chained across
    chunks), matmul2 contracts back to d_out, ScalarE copies PSUM->SBUF
    into per-group staging, SP DMAs each finished group out (one DMA covers
    both o-blocks),
  - chunks are small (128/256) at both ends of the t-range to shrink the
    pipeline fill/drain, 512 in steady state; b0/b1 chunks interleave.
  - the scan-init zero lives in an extra dcols column so the tile framework
    emits no const-materialization prologue barrier.
"""
from contextlib import ExitStack

import numpy as np

import concourse.bass as bass
import concourse.mybir as mybir
import concourse.tile as tile
from concourse import bacc
from concourse.bass_utils import run_bass_kernel_spmd

B, T, D_IN, HID, D_OUT = 16, 4096, 256, 512, 256
NCORES = 8
BPC = B // NCORES          # batch rows per core
ND = D_IN // 128           # 2  d-blocks
NH = HID // 128            # 4  h-blocks
NO = D_OUT // 128          # 2  o-blocks

MODE_DEFAULT = "bf16"

# schedule/tuning knobs (read by _build; cache key includes them)
CFG = dict(
    # per-batch chunk sizes along t (must sum to T)
    chunks=(288, 224, 512, 512, 512, 512, 512, 512, 256, 256),
    # chunk-index groups per output DMA (one staging tile + one DMA each)
    out_groups=((0,), (1,), (2,), (3,), (4,), (5,), (6,), (7,), (8,), (9,)),
    primes=21,             # number of 128-wide prime matmuls
    out_bf16=True,         # stage+DMA outputs as bf16, upcast on host
    xp_bufs=5, op_bufs=3, hs_bufs=16,
    head_cols=288,         # x columns per batch row carried by the head DMAs
    x_second=1024,         # x cols covered by the second piece (from head_cols)
    x_piece=1024,          # bulk x piece size
    pool_scan_units=0,     # first N units: odd-hblk scans go to GPSIMD
    dc_on_sp=False,        # dc as first SP DMA instead of Pool SWDGE
    tail_split=False,      # final unit: stage2 in two col-halves, 2 DMAs
    wi_in_head=False,      # carry all of w_in in head1 (else h23 separate)
)

_cache: dict = {}


def _chunk_plan():
    chunks = []
    start = 0
    for sz in CFG["chunks"]:
        chunks.append((start, sz))
        start += sz
    assert start == T, f"chunks sum {start} != {T}"
    return chunks


def _build(mode: str) -> bass.Bass:
    f32 = mybir.dt.float32
    dt_in = {"bf16": mybir.dt.bfloat16, "f32r": mybir.dt.float32r}.get(
        mode, f32)
    chunks = _chunk_plan()
    groups = CFG["out_groups"]
    assert tuple(sorted(i for g in groups for i in g)) == tuple(range(len(chunks)))
    group_of = {ci: gi for gi, g in enumerate(groups) for ci in g}
    HB = CFG["head_cols"]
    # head1 cols: wi_d0_h01 | wi_d1_h01 | xb0_d0[0:HB] | xb0_d1[0:HB] |
    #             dcols' raw f32 bytes as 2*(NH+1) bf16 cols
    WIH = 1024 if CFG["wi_in_head"] else 512
    H1_COLS = WIH + 2 * HB + 2 * (NH + 1)
    # head2 cols: xb1_d0[0:HB] | xb1_d1[0:HB]
    H2_COLS = 2 * HB

    nc = bacc.Bacc(None, target_bir_lowering=False)

    head1 = nc.declare_dram_parameter("head1", [128, H1_COLS], dt_in,
                                      isOutput=False)
    head2 = nc.declare_dram_parameter("head2", [128, H2_COLS], dt_in,
                                      isOutput=False)
    xT = nc.declare_dram_parameter("xT", [BPC, D_IN, T], dt_in, isOutput=False)
    w_inT = nc.declare_dram_parameter("w_inT", [D_IN, HID], dt_in,
                                      isOutput=False)
    w_outT = nc.declare_dram_parameter("w_outT", [HID, D_OUT], dt_in,
                                       isOutput=False)
    dt_out = dt_in if CFG["out_bf16"] and mode == "bf16" else f32
    out = nc.declare_dram_parameter("out", [BPC, D_OUT, T], dt_out,
                                    isOutput=True)

    with tile.TileContext(nc) as tc, ExitStack() as ctx:
        const_pool = ctx.enter_context(tc.tile_pool(name="const", bufs=1))
        x_pool = ctx.enter_context(tc.tile_pool(name="xt", bufs=BPC * ND))
        o_pool = ctx.enter_context(tc.tile_pool(name="ot", bufs=6))
        hs_pool = ctx.enter_context(tc.tile_pool(name="hs", bufs=CFG["hs_bufs"]))
        xp_psum = ctx.enter_context(
            tc.tile_pool(name="xp", bufs=CFG["xp_bufs"],
                         space=bass.MemorySpace.PSUM))
        op_psum = ctx.enter_context(
            tc.tile_pool(name="op", bufs=CFG["op_bufs"],
                         space=bass.MemorySpace.PSUM))

        # ---- prime matmuls: minimal data deps (one tiny DVE memset); they
        # start the PE p-state ramp clock at t~0. They write into the first
        # real xp tile (cols <=128), which the first real matmul overwrites
        # (start=True) before the scan reads it.
        junk = const_pool.tile([128, 128], dt_in, tag="junk")
        nc.vector.memset(junk[:], 0.0)
        prime_xp = xp_psum.tile([128, 512], f32, name="xp", tag="xp")
        for _ in range(CFG["primes"]):
            nc.tensor.matmul(prime_xp[:, :128], junk[:], junk[:],
                             start=True, stop=True)

        # ---- SBUF tiles
        hd1 = const_pool.tile([128, H1_COLS], dt_in, tag="head1")
        hd2 = const_pool.tile([128, H2_COLS], dt_in, tag="head2")
        wi = const_pool.tile([128, 512], dt_in, tag="wi")       # h23: d0|d1
        wo = const_pool.tile([128, NH * D_OUT], dt_in, tag="wo")  # h-major
        # decay columns: raw f32 bytes shipped inside head1's bf16 transfer
        dcoff = (WIH + 2 * HB) // 2
        dc = hd1[:].bitcast(f32)[:, dcoff:dcoff + NH + 1]
        xt = {}
        for b in range(BPC):
            for dblk in range(ND):
                xt[(b, dblk)] = x_pool.tile([128, T], dt_in, name="xt",
                                            tag="xt")

        def wi_ap(dblk, hblk):
            if CFG["wi_in_head"]:
                return hd1[:, dblk * 512 + hblk * 128:
                           dblk * 512 + (hblk + 1) * 128]
            if hblk < 2:
                return hd1[:, dblk * 256 + hblk * 128:
                           dblk * 256 + (hblk + 1) * 128]
            return wi[:, dblk * 256 + (hblk - 2) * 128:
                      dblk * 256 + (hblk - 1) * 128]

        def x_ap(b, dblk, t0, csz):
            if t0 + csz <= HB:
                hd = (hd1, hd2)[b]
                off = (WIH, 0)[b] + dblk * HB
                return hd[:, off + t0: off + t0 + csz]
            return xt[(b, dblk)][:, t0:t0 + csz]

        # ---- DMA program (emission order = issue order on the SP queue)
        nc.sync.dma_start(hd1[:], head1[:])
        # head2 goes via Pool SWDGE: a parallel issue path that skips the
        # HWDGE device, which serializes SP-queue DMAs 625ns apart
        nc.gpsimd.dma_start(hd2[:], head2[:])
        if not CFG["wi_in_head"]:
            # w_in h-blocks 2/3: rows (dblk*128..), cols [256:512] -> wi d0|d1
            nc.sync.dma_start(
                wi[:].rearrange("p (d h) -> p d h", d=ND),
                w_inT[:].rearrange("(d p) h -> p d h", d=ND)[:, :, 256:])
        # w_out in two halves (hblk 0/1, then 2/3)
        for half in range(2):
            sl = slice(half * 2 * D_OUT, (half + 1) * 2 * D_OUT)
            nc.sync.dma_start(
                wo[:, sl].rearrange("p (h o) -> p h o", h=2),
                w_outT[:].rearrange("(h p) o -> p h o", h=NH)[:, half * 2:half * 2 + 2, :])
        # second x pieces [HB:XS], b0 then b1
        XS = CFG["x_second"]
        for b in range(BPC):
            for dblk in range(ND):
                nc.sync.dma_start(xt[(b, dblk)][:, HB:XS],
                                  xT[b, dblk * 128:(dblk + 1) * 128, HB:XS])
        # bulk x pieces, round-robin b0/b1
        XP = CFG["x_piece"]
        starts = {0: XS, 1: XS}
        while starts[0] < T or starts[1] < T:
            for b in range(BPC):
                s = starts[b]
                if s >= T:
                    continue
                e = min(s + XP, T)
                for dblk in range(ND):
                    nc.sync.dma_start(xt[(b, dblk)][:, s:e],
                                      xT[b, dblk * 128:(dblk + 1) * 128, s:e])
                starts[b] = e

        # ---- compute pipeline
        units = [(b, ci) for ci in range(len(chunks)) for b in range(BPC)]
        prev_hs = {}
        ot_tiles = {}

        fill_xp = {}

        def stage1(b, ci):
            t0, csz = chunks[ci]
            for hblk in range(NH):
                # ci==0 chunks are 256 wide: both batches share one PSUM bank
                # per hblk at different column offsets (halves fill-phase
                # bank pressure; subtile deps keep the scans independent)
                if ci == 0 and csz <= 256:
                    if hblk not in fill_xp:
                        fill_xp[hblk] = (prime_xp if hblk == 0 else
                                         xp_psum.tile([128, 512], f32,
                                                      name="xp", tag="xp"))
                    xp = fill_xp[hblk][:, b * 256: b * 256 + csz]
                elif (b, ci, hblk) == (0, 0, 0):
                    xp = prime_xp[:, :csz]
                else:
                    xp = xp_psum.tile([128, 512], f32,
                                      name="xp", tag="xp")[:, :csz]
                for dblk in range(ND):
                    nc.tensor.matmul(
                        xp,
                        wi_ap(dblk, hblk),
                        x_ap(b, dblk, t0, csz),
                        start=(dblk == 0), stop=(dblk == ND - 1))
                hs = hs_pool.tile([128, 512], dt_in, name="hs", tag="hs")
                init = (dc[:, NH:NH + 1] if ci == 0
                        else prev_hs[(b, ci - 1, hblk)][:, chunks[ci - 1][1] - 1:
                                                        chunks[ci - 1][1]])
                unit_idx = units.index((b, ci))
                eng = (nc.gpsimd if unit_idx < CFG["pool_scan_units"]
                       and hblk % 2 == 1 else nc.vector)
                eng.tensor_tensor_scan(
                    hs[:, :csz], dc[:, hblk:hblk + 1].to_broadcast((128, csz)),
                    xp, init,
                    op0=mybir.AluOpType.mult, op1=mybir.AluOpType.add)
                prev_hs[(b, ci, hblk)] = hs

        def stage2(b, ci):
            t0, csz = chunks[ci]
            gi = group_of[ci]
            g0 = chunks[groups[gi][0]][0]
            gsz = sum(chunks[i][1] for i in groups[gi])
            if (b, gi) not in ot_tiles:
                ot_tiles[(b, gi)] = o_pool.tile([128, 2 * gsz], dt_out,
                                                name="ot", tag="ot")
            ot = ot_tiles[(b, gi)]
            last = (b, ci) == units[-1]
            if last and CFG["tail_split"] and csz % 2 == 0 and gsz == csz:
                # final unit: two column-halves, each with its own copy pair
                # (ACT for oblk0, DVE for oblk1) and its own out DMA, so the
                # drain chain hangs off a half-width copy+transfer
                h2 = csz // 2
                for half in range(2):
                    cs = slice(half * h2, (half + 1) * h2)
                    for oblk in range(NO):
                        op = op_psum.tile([128, 512], f32, name="op", tag="op")
                        for hblk in range(NH):
                            nc.tensor.matmul(
                                op[:, :h2],
                                wo[:, hblk * D_OUT + oblk * 128:
                                   hblk * D_OUT + (oblk + 1) * 128],
                                prev_hs[(b, ci, hblk)][:, cs],
                                start=(hblk == 0), stop=(hblk == NH - 1))
                        dst = ot[:, oblk * csz + half * h2:
                                 oblk * csz + (half + 1) * h2]
                        if oblk == 1:
                            nc.vector.tensor_scalar(
                                dst, op[:, :h2], 0.0, None,
                                op0=mybir.AluOpType.add)
                        else:
                            nc.scalar.copy(dst, op[:, :h2])
                    nc.sync.dma_start(
                        out[b].rearrange("(o p) t -> p o t", o=NO)
                        [:, :, t0 + half * h2: t0 + (half + 1) * h2],
                        ot[:].rearrange("p (o t) -> p o t", o=NO)[:, :, cs])
                return
            for oblk in range(NO):
                op = op_psum.tile([128, 512], f32, name="op", tag="op")
                for hblk in range(NH):
                    nc.tensor.matmul(
                        op[:, :csz],
                        wo[:, hblk * D_OUT + oblk * 128:
                           hblk * D_OUT + (oblk + 1) * 128],
                        prev_hs[(b, ci, hblk)][:, :csz],
                        start=(hblk == 0), stop=(hblk == NH - 1))
                dst = ot[:, oblk * gsz + (t0 - g0):
                         oblk * gsz + (t0 - g0) + csz]
                if last and oblk == 1:
                    nc.vector.tensor_scalar(
                        dst, op[:, :csz], 0.0, None,
                        op0=mybir.AluOpType.add)
                else:
                    nc.scalar.copy(dst, op[:, :csz])
            if ci == groups[gi][-1]:
                nc.sync.dma_start(
                    out[b].rearrange("(o p) t -> p o t", o=NO)[:, :, g0:g0 + gsz],
                    ot[:].rearrange("p (o t) -> p o t", o=NO))

        stage1(*units[0])
        for k in range(len(units) - 1):
            stage1(*units[k + 1])
            stage2(*units[k])
        stage2(*units[-1])

    # Strip the framework's entry-block prologue: four const-AP memsets
    # (f32 0/1, bf16 1, uint8 127 -- none are read by this program) and the
    # all-engine barrier that orders them before the kernel. Engines then
    # branch straight into the tile block ~600ns earlier. The end-of-program
    # drain barrier is untouched (it guarantees output DMAs complete).
    entry = nc.m.functions[0].blocks[0]
    entry.instructions = [
        inst for inst in entry.instructions
        if inst.opcode not in ("Memset", "Drain", "EventSemaphore")
    ]
    nc.compile()
    return nc


def _prep_inputs(x, W_in, W_h, W_out, mode: str):
    npdt = np.float32
    if mode == "bf16":
        import ml_dtypes
        npdt = ml_dtypes.bfloat16
    x = np.asarray(x, np.float32)
    xT = np.ascontiguousarray(np.transpose(x, (0, 2, 1))).astype(npdt)
    w_inT = np.ascontiguousarray(np.asarray(W_in, np.float32).T).astype(npdt)
    w_outT = np.ascontiguousarray(np.asarray(W_out, np.float32).T).astype(npdt)
    d = np.ascontiguousarray(np.diagonal(np.asarray(W_h, np.float32)))
    dcols = np.zeros((128, NH + 1), dtype=np.float32)
    dcols[:, :NH] = d.reshape(NH, 128).T
    import ml_dtypes as _mld
    dcols_bytes = np.ascontiguousarray(dcols).view(_mld.bfloat16)
    HB = CFG["head_cols"]
    in_maps = []
    for c in range(NCORES):
        xc = xT[c * BPC:(c + 1) * BPC]       # [BPC, D_IN, T]
        wn = 512 if CFG["wi_in_head"] else 256
        head1 = np.concatenate(
            [w_inT[0:128, 0:wn], w_inT[128:256, 0:wn],
             xc[0, 0:128, 0:HB], xc[0, 128:256, 0:HB],
             dcols_bytes], axis=1)
        head2 = np.concatenate(
            [xc[1, 0:128, 0:HB], xc[1, 128:256, 0:HB]], axis=1)
        in_maps.append({
            "head1": np.ascontiguousarray(head1),
            "head2": np.ascontiguousarray(head2),
            "xT": np.ascontiguousarray(xc),
            "w_inT": w_inT,
            "w_outT": w_outT,
        })
    return in_maps


def _get_nc(mode: str = MODE_DEFAULT):
    key = (mode, str(sorted(CFG.items())))
    if key not in _cache:
        _cache[key] = _build(mode)
    return _cache[key]


def _run(x, W_in, W_h, W_out, mode: str = MODE_DEFAULT, **spmd_kwargs):
    nc = _get_nc(mode)
    in_maps = _prep_inputs(x, W_in, W_h, W_out, mode)
    res = run_bass_kernel_spmd(nc, in_maps, list(range(NCORES)), **spmd_kwargs)
    parts = [np.transpose(np.asarray(res.results[c]["out"]), (0, 2, 1))
             for c in range(NCORES)]
    full = np.concatenate(parts, axis=0).astype(np.float32)
    return full, res


def kernel(x, W_in, W_h, W_out):
    out, _ = _run(x, W_in, W_h, W_out)
    return out
